# revision 13
# baseline (speedup 1.0000x reference)
"""Adaptive distillation loss on 8 TRN2 NeuronCores.

Math (per row i of logits x[i,:], soft labels s[i,:], temp t_i):
  L1_i  = ln sum_j exp(x_ij)                (logits are N(0,1): no max-shift needed)
  L2_i  = ln sum_j exp(x_ij / t_i)
  ce_i  = L1_i - x[i, y_i]
  kl_i  = sum_j s*ln(s) - (1/t_i) * sum_j s*x + L2_i * sum_j s
  total = 0.5*mean(kl) + 0.5*mean(ce);  avg_temp = mean(t)

Device (data-parallel, 512 rows/core, no collectives): streams x and s tiles
once from HBM; ScalarE does exp(x)+accum, exp(x*invt)+accum, ln(s); VectorE
does fused multiply-reduce for sum(s*x), sum(s*ln s) and sum(s). Host does the
O(B) combination (gather x[i,y_i], logs, means) in float64.
"""

import numpy as np

N_CORES = 8
P = 128            # SBUF partitions per row-block
FD = 2000          # free-dim (columns) per tile

_CACHE = {}


def _build(R, C, fd, reps=1):
    """Build the per-core Bass graph for an [R, C] shard (R rows, C cols).

    reps > 1 repeats the whole compute serially inside the NEFF (for
    benchmarking via the wall-clock slope between reps=1 and reps=N)."""
    import concourse.bacc as bacc
    import concourse.tile as tile
    from concourse import mybir

    AF = mybir.ActivationFunctionType
    OP = mybir.AluOpType
    f32 = mybir.dt.float32
    rb = R // P          # row blocks
    nt = C // fd         # column tiles per block

    nc = bacc.Bacc("TRN2", target_bir_lowering=False, debug=False,
                   num_devices=N_CORES)
    x_d = nc.dram_tensor("logits", [R, C], f32, kind="ExternalInput").ap()
    s_d = nc.dram_tensor("soft", [R, C], f32, kind="ExternalInput").ap()
    invt_d = nc.dram_tensor("invt", [rb, P, 1], f32, kind="ExternalInput").ap()
    # out[b, q, p]: q = 0:sum_exp  1:sum_exp_t  2:dot  3:ent  4:sum_s
    out_d = nc.dram_tensor("out", [rb, 5, P, 1], f32, kind="ExternalOutput").ap()

    with tile.TileContext(nc) as tc:
        with (
            tc.tile_pool(name="xp", bufs=4) as xp,
            tc.tile_pool(name="sp", bufs=4) as sp,
            tc.tile_pool(name="lsp", bufs=3) as lsp,
            tc.tile_pool(name="ga", bufs=2) as gap,    # ACT garbage outs
            tc.tile_pool(name="gv", bufs=3) as gvp,    # DVE garbage outs
            tc.tile_pool(name="acc", bufs=2) as accp,
            tc.tile_pool(name="small", bufs=2) as smallp,
        ):
            lnbias = smallp.tile([P, 1], f32, tag="lnbias", name="lnbias")
            nc.vector.memset(lnbias, 1e-38)
            for b in [b for _ in range(reps) for b in range(rb)]:
                invt_t = smallp.tile([P, 1], f32, tag="invt")
                nc.sync.dma_start(out=invt_t, in_=invt_d[b])
                accs = [accp.tile([P, nt], f32, tag=f"acc{q}", name=f"acc{q}_{b}")
                        for q in range(5)]
                for t in range(nt):
                    rows = slice(b * P, (b + 1) * P)
                    cols = slice(t * fd, (t + 1) * fd)
                    xt = xp.tile([P, fd], f32, tag="x")
                    nc.sync.dma_start(out=xt, in_=x_d[rows, cols])
                    st = sp.tile([P, fd], f32, tag="s")
                    nc.sync.dma_start(out=st, in_=s_d[rows, cols])

                    e1 = gap.tile([P, fd], f32, tag="ga")
                    nc.scalar.activation(out=e1, in_=xt, func=AF.Exp,
                                         accum_out=accs[0][:, t:t + 1])
                    e2 = gap.tile([P, fd], f32, tag="ga")
                    nc.scalar.activation(out=e2, in_=xt, func=AF.Exp,
                                         scale=invt_t,
                                         accum_out=accs[1][:, t:t + 1])
                    ls = lsp.tile([P, fd], f32, tag="ls")
                    # bias guards s==0: ln(0+1e-38) finite, then s*ls -> 0
                    nc.scalar.activation(out=ls, in_=st, func=AF.Ln,
                                         bias=lnbias)

                    p1 = gvp.tile([P, fd], f32, tag="gv")
                    nc.vector.scalar_tensor_tensor(
                        out=p1, in0=xt, scalar=1.0, in1=st,
                        op0=OP.mult, op1=OP.mult,
                        accum_out=accs[2][:, t:t + 1])
                    p2 = gvp.tile([P, fd], f32, tag="gv")
                    nc.vector.scalar_tensor_tensor(
                        out=p2, in0=ls, scalar=1.0, in1=st,
                        op0=OP.mult, op1=OP.mult,
                        accum_out=accs[3][:, t:t + 1])
                    p3 = gvp.tile([P, fd], f32, tag="gv")
                    nc.vector.tensor_scalar(
                        out=p3, in0=st, scalar1=1.0, scalar2=None,
                        op0=OP.mult, op1=OP.add,
                        accum_out=accs[4][:, t:t + 1])
                for q in range(5):
                    red = smallp.tile([P, 1], f32, tag=f"red{q}")
                    nc.vector.tensor_reduce(out=red, in_=accs[q],
                                            axis=mybir.AxisListType.X,
                                            op=OP.add)
                    nc.sync.dma_start(out=out_d[b, q], in_=red)
    nc.compile()
    return nc


def _get_nc(R, C, fd=FD):
    key = (R, C, fd)
    if key not in _CACHE:
        _CACHE[key] = _build(R, C, fd)
    return _CACHE[key]


def _temps_np(conf):
    c = conf.astype(np.float32)
    low = np.minimum(np.float32(2.5) + (np.float32(0.6) - c) * np.float32(2.0),
                     np.float32(3.0)).astype(np.float32)
    return np.where(c > np.float32(0.9), np.float32(1.5),
                    np.where(c > np.float32(0.6), np.float32(2.0),
                             low)).astype(np.float32)


def run(inputs, trace=False):
    """Returns ((total, ce, kl, avg_temp), BassKernelResults)."""
    from concourse import bass_utils

    logits = np.ascontiguousarray(np.asarray(inputs["logits"], np.float32))
    soft = np.ascontiguousarray(np.asarray(inputs["soft_labels"], np.float32))
    hard = np.asarray(inputs["hard_labels"])
    conf = np.asarray(inputs["confidences"], np.float32)

    B, C = logits.shape
    R = B // N_CORES
    rb = R // P

    temps = _temps_np(conf)
    invt = (np.float32(1.0) / temps).astype(np.float32)

    nc = _get_nc(R, C)
    in_maps = []
    for c in range(N_CORES):
        sl = slice(c * R, (c + 1) * R)
        in_maps.append({
            "logits": logits[sl],
            "soft": soft[sl],
            "invt": np.ascontiguousarray(invt[sl].reshape(rb, P, 1)),
        })
    res = bass_utils.run_bass_kernel_spmd(
        nc, in_maps, core_ids=list(range(N_CORES)), trace=trace)

    out = np.stack([r["out"] for r in res.results])      # [8, rb, 5, P, 1]
    vals = out.reshape(N_CORES, rb, 5, P).transpose(2, 0, 1, 3).reshape(5, B)
    sum1, sum2, dot, ent, sums = vals.astype(np.float64)

    L1 = np.log(sum1)
    L2 = np.log(sum2)
    picked = logits[np.arange(B), hard].astype(np.float64)
    ce_rows = L1 - picked
    kl_rows = ent - invt.astype(np.float64) * dot + L2 * sums
    ce = ce_rows.mean()
    kl = kl_rows.mean()
    total = 0.5 * kl + 0.5 * ce
    avg_t = temps.astype(np.float64).mean()
    outs = (np.float32(total), np.float32(ce), np.float32(kl),
            np.float32(avg_t))
    return outs, res


def kernel(**inputs):
    return run(inputs, trace=False)[0]


def _prep_in_maps(inputs):
    logits = np.ascontiguousarray(np.asarray(inputs["logits"], np.float32))
    soft = np.ascontiguousarray(np.asarray(inputs["soft_labels"], np.float32))
    conf = np.asarray(inputs["confidences"], np.float32)
    B, C = logits.shape
    R = B // N_CORES
    rb = R // P
    temps = _temps_np(conf)
    invt = (np.float32(1.0) / temps).astype(np.float32)
    in_maps = []
    for c in range(N_CORES):
        sl = slice(c * R, (c + 1) * R)
        in_maps.append({
            "logits": logits[sl],
            "soft": soft[sl],
            "invt": np.ascontiguousarray(invt[sl].reshape(rb, P, 1)),
        })
    return in_maps, R, C


def _make_runner(nc, in_maps):
    """Jitted single-bind runner over device-resident sharded inputs.
    Returns a zero-arg callable executing the NEFF once across 8 cores."""
    import jax
    from jax.sharding import Mesh, PartitionSpec, NamedSharding
    from jax.experimental.shard_map import shard_map
    from concourse import bass2jax, mybir

    bass2jax.install_neuronx_cc_hook()
    partition_name = (nc.partition_id_tensor.name
                      if nc.partition_id_tensor else None)
    in_names, out_names, out_avals, zero_outs = [], [], [], []
    for alloc in nc.m.functions[0].allocations:
        if not isinstance(alloc, mybir.MemoryLocationSet):
            continue
        name = alloc.memorylocations[0].name
        if alloc.kind == "ExternalInput":
            if name != partition_name:
                in_names.append(name)
        elif alloc.kind == "ExternalOutput":
            shape = tuple(alloc.tensor_shape)
            dtype = mybir.dt.np(alloc.dtype)
            out_avals.append(jax.core.ShapedArray(shape, dtype))
            out_names.append(name)
            zero_outs.append(np.zeros(shape, dtype))
    n_params = len(in_names)
    bind_in_names = tuple(in_names + out_names +
                          ([partition_name] if partition_name else []))

    def _body(*args):
        operands = list(args)
        if partition_name:
            operands.append(bass2jax.partition_id_tensor())
        outs = bass2jax._bass_exec_p.bind(
            *operands,
            out_avals=tuple(out_avals),
            in_names=bind_in_names,
            out_names=tuple(out_names),
            lowering_input_output_aliases=(),
            sim_require_finite=True,
            sim_require_nnan=True,
            nc=nc,
        )
        return tuple(outs)

    devices = jax.devices()[:N_CORES]
    mesh = Mesh(np.asarray(devices), ("core",))
    n_outs = len(out_names)
    fn = jax.jit(shard_map(_body, mesh=mesh,
                           in_specs=(PartitionSpec("core"),) * (n_params + n_outs),
                           out_specs=(PartitionSpec("core"),) * n_outs,
                           check_rep=False))
    sh = NamedSharding(mesh, PartitionSpec("core"))
    per_core = [[np.asarray(m[name]) for name in in_names] for m in in_maps]
    dev_in = [jax.device_put(
        np.concatenate([per_core[c][i] for c in range(N_CORES)], 0), sh)
        for i in range(n_params)]
    dev_zeros = [jax.device_put(
        np.zeros((N_CORES * z.shape[0], *z.shape[1:]), z.dtype), sh)
        for z in zero_outs]

    def call():
        return jax.block_until_ready(fn(*dev_in, *dev_zeros))
    return call


def bench(inputs, reps=5, builder=None, tries=8):
    """Per-execution HW time (ns) via the wall-clock slope between NEFFs
    that repeat the compute 1x and `reps`x internally (dispatch overhead
    cancels); inputs stay device-resident."""
    import time

    builder = builder or _build
    in_maps, R, C = _prep_in_maps(inputs)

    def timed(k):
        nc = builder(R, C, FD, reps=k)
        call = _make_runner(nc, in_maps)
        call()  # compile + warm
        best = float("inf")
        for _ in range(tries):
            t0 = time.perf_counter()
            call()
            best = min(best, time.perf_counter() - t0)
        return best

    t1 = timed(1)
    tk = timed(reps)
    per_exec_ns = (tk - t1) / (reps - 1) * 1e9
    print(f"bench: t1={t1*1e3:.2f}ms t{reps}={tk*1e3:.2f}ms "
          f"-> {per_exec_ns:.0f} ns/exec")
    return per_exec_ns


# revision 15
# speedup vs baseline: 1.2639x; 1.2639x over previous
"""Adaptive distillation loss on 8 TRN2 NeuronCores.

Math (per row i of logits x[i,:], soft labels s[i,:], temp t_i):
  L1_i  = ln sum_j exp(x_ij)                (logits are N(0,1): no max-shift needed)
  L2_i  = ln sum_j exp(x_ij / t_i)
  ce_i  = L1_i - x[i, y_i]
  kl_i  = sum_j s*ln(s) - (1/t_i) * sum_j s*x + L2_i * sum_j s
  total = 0.5*mean(kl) + 0.5*mean(ce);  avg_temp = mean(t)

Device (data-parallel, 512 rows/core, no collectives): streams x and s tiles
once from HBM; ScalarE does exp(x)+accum, exp(x*invt)+accum, ln(s); VectorE
does fused multiply-reduce for sum(s*x), sum(s*ln s) and sum(s). Host does the
O(B) combination (gather x[i,y_i], logs, means) in float64.
"""

import numpy as np

N_CORES = 8
P = 128            # SBUF partitions per row-block
FD = 2000          # free-dim (columns) per tile

_CACHE = {}


def _build(R, C, fd, reps=1):
    """Build the per-core Bass graph for an [R, C] shard (R rows, C cols).

    reps > 1 repeats the whole compute serially inside the NEFF (for
    benchmarking via the wall-clock slope between reps=1 and reps=N)."""
    import concourse.bacc as bacc
    import concourse.tile as tile
    from concourse import mybir

    AF = mybir.ActivationFunctionType
    OP = mybir.AluOpType

    # Both Exp and Ln live in the "natural_log_exp_and_others" ACT table
    # set, but the table-load planner greedily picks the first set holding
    # each function, inserting a table switch (~1.3us) between every Exp
    # and Ln. Strip Exp/Ln from the other sets (positional set ids
    # preserved) so both resolve to the combined set -> one load total.
    if not getattr(bacc, "_act_tables_patched", False):
        _orig_tables = bacc.get_activation_tables

        def _patched(arch):
            out = {}
            for name, funcs in _orig_tables(arch).items():
                if name != "natural_log_exp_and_others":
                    funcs = funcs - {AF.Exp, AF.Ln}
                out[name] = funcs
            return out

        bacc.get_activation_tables = _patched
        bacc._act_tables_patched = True
    f32 = mybir.dt.float32
    rb = R // P          # row blocks
    nt = C // fd         # column tiles per block

    nc = bacc.Bacc("TRN2", target_bir_lowering=False, debug=False,
                   num_devices=N_CORES)
    x_d = nc.dram_tensor("logits", [R, C], f32, kind="ExternalInput").ap()
    s_d = nc.dram_tensor("soft", [R, C], f32, kind="ExternalInput").ap()
    invt_d = nc.dram_tensor("invt", [rb, P, 1], f32, kind="ExternalInput").ap()
    # out[b, q, p]: q = 0:sum_exp  1:sum_exp_t  2:dot  3:ent  4:sum_s
    out_d = nc.dram_tensor("out", [rb, 5, P, 1], f32, kind="ExternalOutput").ap()

    with tile.TileContext(nc) as tc:
        with (
            tc.tile_pool(name="xp", bufs=4) as xp,
            tc.tile_pool(name="sp", bufs=4) as sp,
            tc.tile_pool(name="lsp", bufs=3) as lsp,
            tc.tile_pool(name="ga", bufs=2) as gap,    # ACT garbage outs
            tc.tile_pool(name="gv", bufs=3) as gvp,    # DVE garbage outs
            tc.tile_pool(name="acc", bufs=2) as accp,
            tc.tile_pool(name="small", bufs=2) as smallp,
        ):
            lnbias = smallp.tile([P, 1], f32, tag="lnbias", name="lnbias")
            nc.vector.memset(lnbias, 1e-38)
            for b in [b for _ in range(reps) for b in range(rb)]:
                invt_t = smallp.tile([P, 1], f32, tag="invt")
                nc.sync.dma_start(out=invt_t, in_=invt_d[b])
                accs = [accp.tile([P, nt], f32, tag=f"acc{q}", name=f"acc{q}_{b}")
                        for q in range(5)]
                for t in range(nt):
                    rows = slice(b * P, (b + 1) * P)
                    cols = slice(t * fd, (t + 1) * fd)
                    xt = xp.tile([P, fd], f32, tag="x")
                    nc.sync.dma_start(out=xt, in_=x_d[rows, cols])
                    st = sp.tile([P, fd], f32, tag="s")
                    nc.sync.dma_start(out=st, in_=s_d[rows, cols])

                    e1 = gap.tile([P, fd], f32, tag="ga")
                    nc.scalar.activation(out=e1, in_=xt, func=AF.Exp,
                                         accum_out=accs[0][:, t:t + 1])
                    e2 = gap.tile([P, fd], f32, tag="ga")
                    nc.scalar.activation(out=e2, in_=xt, func=AF.Exp,
                                         scale=invt_t,
                                         accum_out=accs[1][:, t:t + 1])
                    ls = lsp.tile([P, fd], f32, tag="ls")
                    # bias guards s==0: ln(0+1e-38) finite, then s*ls -> 0
                    nc.scalar.activation(out=ls, in_=st, func=AF.Ln,
                                         bias=lnbias)

                    p1 = gvp.tile([P, fd], f32, tag="gv")
                    nc.vector.scalar_tensor_tensor(
                        out=p1, in0=xt, scalar=1.0, in1=st,
                        op0=OP.mult, op1=OP.mult,
                        accum_out=accs[2][:, t:t + 1])
                    p2 = gvp.tile([P, fd], f32, tag="gv")
                    nc.vector.scalar_tensor_tensor(
                        out=p2, in0=ls, scalar=1.0, in1=st,
                        op0=OP.mult, op1=OP.mult,
                        accum_out=accs[3][:, t:t + 1])
                    p3 = gvp.tile([P, fd], f32, tag="gv")
                    nc.vector.tensor_scalar(
                        out=p3, in0=st, scalar1=1.0, scalar2=None,
                        op0=OP.mult, op1=OP.add,
                        accum_out=accs[4][:, t:t + 1])
                for q in range(5):
                    red = smallp.tile([P, 1], f32, tag=f"red{q}")
                    nc.vector.tensor_reduce(out=red, in_=accs[q],
                                            axis=mybir.AxisListType.X,
                                            op=OP.add)
                    nc.sync.dma_start(out=out_d[b, q], in_=red)
    nc.compile()
    return nc


def _get_nc(R, C, fd=FD):
    key = (R, C, fd)
    if key not in _CACHE:
        _CACHE[key] = _build(R, C, fd)
    return _CACHE[key]


def _temps_np(conf):
    c = conf.astype(np.float32)
    low = np.minimum(np.float32(2.5) + (np.float32(0.6) - c) * np.float32(2.0),
                     np.float32(3.0)).astype(np.float32)
    return np.where(c > np.float32(0.9), np.float32(1.5),
                    np.where(c > np.float32(0.6), np.float32(2.0),
                             low)).astype(np.float32)


def run(inputs, trace=False):
    """Returns ((total, ce, kl, avg_temp), BassKernelResults)."""
    from concourse import bass_utils

    logits = np.ascontiguousarray(np.asarray(inputs["logits"], np.float32))
    soft = np.ascontiguousarray(np.asarray(inputs["soft_labels"], np.float32))
    hard = np.asarray(inputs["hard_labels"])
    conf = np.asarray(inputs["confidences"], np.float32)

    B, C = logits.shape
    R = B // N_CORES
    rb = R // P

    temps = _temps_np(conf)
    invt = (np.float32(1.0) / temps).astype(np.float32)

    nc = _get_nc(R, C)
    in_maps = []
    for c in range(N_CORES):
        sl = slice(c * R, (c + 1) * R)
        in_maps.append({
            "logits": logits[sl],
            "soft": soft[sl],
            "invt": np.ascontiguousarray(invt[sl].reshape(rb, P, 1)),
        })
    res = bass_utils.run_bass_kernel_spmd(
        nc, in_maps, core_ids=list(range(N_CORES)), trace=trace)

    out = np.stack([r["out"] for r in res.results])      # [8, rb, 5, P, 1]
    vals = out.reshape(N_CORES, rb, 5, P).transpose(2, 0, 1, 3).reshape(5, B)
    sum1, sum2, dot, ent, sums = vals.astype(np.float64)

    L1 = np.log(sum1)
    L2 = np.log(sum2)
    picked = logits[np.arange(B), hard].astype(np.float64)
    ce_rows = L1 - picked
    kl_rows = ent - invt.astype(np.float64) * dot + L2 * sums
    ce = ce_rows.mean()
    kl = kl_rows.mean()
    total = 0.5 * kl + 0.5 * ce
    avg_t = temps.astype(np.float64).mean()
    outs = (np.float32(total), np.float32(ce), np.float32(kl),
            np.float32(avg_t))
    return outs, res


def kernel(**inputs):
    return run(inputs, trace=False)[0]


def _prep_in_maps(inputs):
    logits = np.ascontiguousarray(np.asarray(inputs["logits"], np.float32))
    soft = np.ascontiguousarray(np.asarray(inputs["soft_labels"], np.float32))
    conf = np.asarray(inputs["confidences"], np.float32)
    B, C = logits.shape
    R = B // N_CORES
    rb = R // P
    temps = _temps_np(conf)
    invt = (np.float32(1.0) / temps).astype(np.float32)
    in_maps = []
    for c in range(N_CORES):
        sl = slice(c * R, (c + 1) * R)
        in_maps.append({
            "logits": logits[sl],
            "soft": soft[sl],
            "invt": np.ascontiguousarray(invt[sl].reshape(rb, P, 1)),
        })
    return in_maps, R, C


def _make_runner(nc, in_maps):
    """Jitted single-bind runner over device-resident sharded inputs.
    Returns a zero-arg callable executing the NEFF once across 8 cores."""
    import jax
    from jax.sharding import Mesh, PartitionSpec, NamedSharding
    from jax.experimental.shard_map import shard_map
    from concourse import bass2jax, mybir

    bass2jax.install_neuronx_cc_hook()
    partition_name = (nc.partition_id_tensor.name
                      if nc.partition_id_tensor else None)
    in_names, out_names, out_avals, zero_outs = [], [], [], []
    for alloc in nc.m.functions[0].allocations:
        if not isinstance(alloc, mybir.MemoryLocationSet):
            continue
        name = alloc.memorylocations[0].name
        if alloc.kind == "ExternalInput":
            if name != partition_name:
                in_names.append(name)
        elif alloc.kind == "ExternalOutput":
            shape = tuple(alloc.tensor_shape)
            dtype = mybir.dt.np(alloc.dtype)
            out_avals.append(jax.core.ShapedArray(shape, dtype))
            out_names.append(name)
            zero_outs.append(np.zeros(shape, dtype))
    n_params = len(in_names)
    bind_in_names = tuple(in_names + out_names +
                          ([partition_name] if partition_name else []))

    def _body(*args):
        operands = list(args)
        if partition_name:
            operands.append(bass2jax.partition_id_tensor())
        outs = bass2jax._bass_exec_p.bind(
            *operands,
            out_avals=tuple(out_avals),
            in_names=bind_in_names,
            out_names=tuple(out_names),
            lowering_input_output_aliases=(),
            sim_require_finite=True,
            sim_require_nnan=True,
            nc=nc,
        )
        return tuple(outs)

    devices = jax.devices()[:N_CORES]
    mesh = Mesh(np.asarray(devices), ("core",))
    n_outs = len(out_names)
    fn = jax.jit(shard_map(_body, mesh=mesh,
                           in_specs=(PartitionSpec("core"),) * (n_params + n_outs),
                           out_specs=(PartitionSpec("core"),) * n_outs,
                           check_rep=False))
    sh = NamedSharding(mesh, PartitionSpec("core"))
    per_core = [[np.asarray(m[name]) for name in in_names] for m in in_maps]
    dev_in = [jax.device_put(
        np.concatenate([per_core[c][i] for c in range(N_CORES)], 0), sh)
        for i in range(n_params)]
    dev_zeros = [jax.device_put(
        np.zeros((N_CORES * z.shape[0], *z.shape[1:]), z.dtype), sh)
        for z in zero_outs]

    def call():
        return jax.block_until_ready(fn(*dev_in, *dev_zeros))
    return call


def bench(inputs, reps=9, builder=None, tries=25, fd=None):
    """Per-execution HW time (ns) via the wall-clock slope between NEFFs
    that repeat the compute 1x and `reps`x internally (dispatch overhead
    cancels); inputs stay device-resident; samples interleaved to cancel
    drift."""
    import time

    builder = builder or _build
    in_maps, R, C = _prep_in_maps(inputs)

    calls = {}
    for k in (1, reps):
        nc = builder(R, C, fd or FD, reps=k)
        calls[k] = _make_runner(nc, in_maps)
        calls[k]()  # compile + warm

    samples = {1: [], reps: []}
    for _ in range(tries):
        for k in (1, reps):
            t0 = time.perf_counter()
            calls[k]()
            samples[k].append(time.perf_counter() - t0)
    t1, tk = min(samples[1]), min(samples[reps])
    per_exec_ns = (tk - t1) / (reps - 1) * 1e9
    print(f"bench: t1={t1*1e3:.2f}ms t{reps}={tk*1e3:.2f}ms "
          f"-> {per_exec_ns:.0f} ns/exec")
    return per_exec_ns
